# revision 45
# baseline (speedup 1.0000x reference)
"""Trainium2 Bass kernel for nn_DocREModel_Triangle (DocRE block-bilinear model).

Strategy (8 NeuronCores, single SPMD NEFF):
  Phase 1 (pair-parallel): core c owns batch b=c//4 and entity rows
  i in [6*(c%4), 6*(c%4)+6) -> 144 (i,j) pairs. Computes, in a transposed
  layout (feature dim on partitions, pairs on the free dim): mention
  gather + logsumexp entity embeddings, attention-row gather + mention-sum
  (fused into selector matmuls whose outputs stay packed in PSUM),
  pairwise head-products (DVE mul + halving add-tree over heads),
  l-normalization, context vectors rs^T, and the two tanh extractors
  hs^T/ts^T [768, 144] bf16. The hpart/tpart broadcast-adds are folded
  into the extractor PSUM accumulation as tiny selector matmuls.
  Collectives: AllGather replicates ts^T, AllToAll redistributes hs^T by
  s-slices. Both cc buffers are partition-major [128, 6, 144] so the
  bounce DMAs are straight copies; a host-side column permutation of Wh
  makes the A2A 16-partition shards line up with s-slices.
  Phase 2 (contraction-parallel): core c holds Wp rows (k, s in [8c,8c+8),
  t) resident in SBUF (bf16, 9.4 MB) and accumulates feature^T over its
  6144 (k,s,t) rows for all 1152 pairs in 4 pair-chunks of 288. The hs
  operand is replicated to 64 partitions by stride-0-partition DMA, ts is
  loaded once [128, 12, 1152] during the A2A. Matmuls run sp-inner so 4
  consecutive MMs accumulate into the same PSUM bank. The classifier is
  C-padded to 104 and ReduceScattered: core c keeps logit rows
  [13c, 13c+13); the host concatenates the 8 slices.
"""

import numpy as np
import ml_dtypes

bf16 = ml_dtypes.bfloat16

B, L, H, NH = 2, 512, 768, 12
NE, NM = 24, 4
E, BS, C = 768, 64, 97
K = E // BS                      # 12 blocks
NCORE = 8
IPC = NE // 4                    # 6 i-rows per core (4 cores per batch elem)
PL = IPC * NE                    # 144 local pairs
NP = B * NE * NE                 # 1152 global pairs
SL = 64 // NCORE                 # 8 s-values per core
KST = K * SL * BS                # 6144 contraction rows per core
NKT = KST // 128                 # 48 contraction tiles
NCHUNK = 4                       # pair chunks of 288 (= 2 pair-blocks)
CW = NP // NCHUNK                # 288
CP = 104                         # class count padded to a multiple of 8
CSL = CP // NCORE                # 13 logit rows per core after ReduceScatter

# hs column permutation: extractor output column c = Et*128 + p0 holds
# original dim e = k*64 + s with k = 6*(p0%2) + Et, s = 8*(p0//16) +
# (p0%16)//2.  The p-major cc buffer [128, 6, PL] then has A2A shard j
# (partitions 16j..16j+16) == s-slice [8j, 8j+8), and the phase-2 hs
# broadcast reads are 3-dim APs.
_PERM2 = np.empty(E, dtype=np.int64)
for _c in range(E):
    _p0, _Et = _c % 128, _c // 128
    _k = 6 * (_p0 % 2) + _Et
    _s = 8 * (_p0 // 16) + (_p0 % 16) // 2
    _PERM2[_c] = _k * 64 + _s


def _host_prep(inputs):
    """Build the 8 per-core input maps from the full inputs."""
    seq = np.ascontiguousarray(inputs["sequence_output"], dtype=np.float32)
    att = np.ascontiguousarray(inputs["attention"], dtype=np.float32)
    Wh = np.asarray(inputs["Wh"], dtype=np.float32)
    bh = np.asarray(inputs["bh"], dtype=np.float32)
    Wt = np.asarray(inputs["Wt"], dtype=np.float32)
    bt = np.asarray(inputs["bt"], dtype=np.float32)
    Wp = np.asarray(inputs["Wp"], dtype=np.float32)
    Wc = np.asarray(inputs["Wc"], dtype=np.float32)
    bc = np.asarray(inputs["bc"], dtype=np.float32)
    mpos = np.asarray(inputs["mention_pos"]).astype(np.int64)

    wh1p = np.ascontiguousarray(Wh[:H][:, _PERM2].astype(bf16))
    wh2p = np.ascontiguousarray(Wh[H:][:, _PERM2].astype(bf16))
    wt1 = np.ascontiguousarray(Wt[:H].astype(bf16))
    wt2 = np.ascontiguousarray(Wt[H:].astype(bf16))
    bh_p = np.ascontiguousarray(bh[_PERM2].reshape(6, 128).T.astype(np.float32))
    bt_t = np.ascontiguousarray(bt.reshape(6, 128).T.astype(np.float32))
    wc_pad = np.zeros((H, CP), dtype=np.float32)
    wc_pad[:, :C] = Wc
    wc_bf = np.ascontiguousarray(wc_pad.astype(bf16))
    bc_pad = np.zeros((CP,), dtype=np.float32)
    bc_pad[:C] = bc
    wp4 = Wp.reshape(K, 64, BS, H)

    # per-pair mask row (zero for self-pairs i == j)
    mrow = np.ones((NP,), dtype=np.float32)
    for c in range(NCORE):
        for il in range(IPC):
            ig = (c % 4) * IPC + il
            mrow[c * PL + il * NE + ig] = 0.0

    in_maps = []
    for c in range(NCORE):
        b = c // 4
        i0 = (c % 4) * IPC
        ents = list(range(NE)) + list(range(i0, i0 + IPC))  # 24 j-side + 6 i-side

        # mention logsumexp + first-layer projections on host (tiny: 56M MACs)
        ee = np.log(np.exp(seq[b][mpos[b][ents]].astype(np.float64)).sum(axis=1))
        hpTt_in = np.ascontiguousarray(
            (ee[NE:] @ Wh[:H].astype(np.float64))[:, _PERM2].astype(bf16))
        tpTt_in = np.ascontiguousarray(
            (ee[:NE] @ Wt[:H].astype(np.float64)).astype(bf16))

        # host-gathered attention rows: [128, NH, L] bf16,
        # rows = 4m x 30ents (+ 8 pad) per head
        ai = np.zeros((128, NH), dtype=np.int32)
        for h in range(NH):
            for m in range(NM):
                for e_i, ent in enumerate(ents):
                    ai[m * 30 + e_i, h] = h * L + mpos[b, ent, m]
        att_gb = np.ascontiguousarray(
            att[b].reshape(NH * L, L)[ai.T].transpose(1, 0, 2).astype(bf16))

        wp_sl = np.ascontiguousarray(
            wp4[:, SL * c : SL * (c + 1)].reshape(KST, H).astype(bf16)
        )

        in_maps.append(
            {
                "seq_bf": np.ascontiguousarray(seq[b].astype(bf16)),
                "att_gb": att_gb,
                "hpTt_in": hpTt_in,
                "tpTt_in": tpTt_in,
                "wh2p": wh2p,
                "wt2": wt2,
                "bh_p": bh_p,
                "bt_t": bt_t,
                "wp_sl": wp_sl,
                "wc_bf": wc_bf,
                "bc_sl": np.ascontiguousarray(
                    bc_pad[CSL * c : CSL * (c + 1)].reshape(CSL, 1)),
                "mask_sl": np.ascontiguousarray(
                    np.tile(mrow[None, :], (CSL, 1))),
            }
        )
    return in_maps


def _build_consts():
    # selectors summing the mention rows per entity during the attention /
    # mention transpose-matmuls (the /4 mean cancels in the normalization)
    S = np.zeros((120, 30), dtype=bf16)
    for m in range(NM):
        for e_i in range(30):
            S[m * 30 + e_i, e_i] = 1.0
    S2 = np.zeros((128, 30), dtype=bf16)
    for m in range(NM):
        for e_i in range(30):
            S2[m * 32 + e_i, e_i] = 1.0
    ones_bf = np.ones((128, 1), dtype=bf16)
    ones_row = np.ones((1, 128), dtype=np.float32)
    # pair selectors: selI picks hpart row il for pair (il, j), selJ picks
    # tpart row j; used as 6-/24-deep matmuls folded into the extractors
    selI = np.zeros((IPC, PL), dtype=bf16)
    selJ = np.zeros((NE, PL), dtype=bf16)
    for il in range(IPC):
        for j in range(NE):
            selI[il, il * NE + j] = 1.0
            selJ[j, il * NE + j] = 1.0
    return S, S2, ones_bf, ones_row, selI, selJ


def build_bass():
    import concourse.bass as bass
    import concourse.mybir as mybir
    import concourse.tile as tile
    from concourse.bacc import Bacc

    f32 = mybir.dt.float32
    bft = mybir.dt.bfloat16
    i32 = mybir.dt.int32
    AF = mybir.ActivationFunctionType
    ALU = mybir.AluOpType

    nc = Bacc("TRN2", num_devices=NCORE)

    # ---- I/O ----
    seq_bf = nc.dram_tensor("seq_bf", [L, H], bft, kind="ExternalInput")
    att_gb = nc.dram_tensor("att_gb", [128, NH, L], bft, kind="ExternalInput")
    hpTt_in = nc.dram_tensor("hpTt_in", [IPC, E], bft, kind="ExternalInput")
    tpTt_in = nc.dram_tensor("tpTt_in", [NE, E], bft, kind="ExternalInput")
    wh2p = nc.dram_tensor("wh2p", [H, E], bft, kind="ExternalInput")
    wt2 = nc.dram_tensor("wt2", [H, E], bft, kind="ExternalInput")
    bh_p = nc.dram_tensor("bh_p", [128, 6], f32, kind="ExternalInput")
    bt_t = nc.dram_tensor("bt_t", [128, 6], f32, kind="ExternalInput")
    wp_sl = nc.dram_tensor("wp_sl", [KST, H], bft, kind="ExternalInput")
    wc_bf = nc.dram_tensor("wc_bf", [H, CP], bft, kind="ExternalInput")
    bc_sl = nc.dram_tensor("bc_sl", [CSL, 1], f32, kind="ExternalInput")
    mask_sl = nc.dram_tensor("mask_sl", [CSL, NP], f32, kind="ExternalInput")
    out_lgT = nc.dram_tensor("out_lgT", [CSL, NP], f32, kind="ExternalOutput")

    S_np, S2_np, ones_np, onesrow_np, selI_np, selJ_np = _build_consts()
    S_dr = nc.inline_tensor(S_np, "sel_const")
    S2_dr = nc.inline_tensor(S2_np, "s2_const")
    ones_dr = nc.inline_tensor(ones_np, "ones_const")
    onesrow_dr = nc.inline_tensor(onesrow_np, "onesrow_const")
    selI_dr = nc.inline_tensor(selI_np, "selI_const")
    selJ_dr = nc.inline_tensor(selJ_np, "selJ_const")

    # collective buffers, partition-major so bounce DMAs are straight copies
    hs_cc_in = nc.dram_tensor("hs_cc_in", [128, 6, PL], bft)
    hs_cc_out = nc.dram_tensor("hs_cc_out", [128, 6, PL], bft)
    ts_cc_in = nc.dram_tensor("ts_cc_in", [128, 6, PL], bft)
    ts_cc_o = nc.dram_tensor("ts_cc_o", [NCORE, 128, 6, PL], bft, addr_space="Shared")
    lg_cc_in = nc.dram_tensor("lg_cc_in", [NCHUNK, CP, CW], bft)
    lg_cc_out = nc.dram_tensor("lg_cc_out", [NCHUNK, CSL, CW], bft)
    groups = [list(range(NCORE))]

    with tile.TileContext(nc) as tc:
        with (
            tc.tile_pool(name="gpool", bufs=1) as gpool,
            tc.tile_pool(name="persist", bufs=1) as persist,
        ):
            # ---------- whole-kernel-lifetime weights / constants ----------
            # wp split in two so the lhsT partition pitch stays under 64 KiB
            # (a single [128, 48, 768] bf16 tile has 73.7 KB pitch, which
            # disables the fast-weight-load path: 108 ns vs 62 ns LDWEIGHTS)
            wp_a = gpool.tile([128, NKT // 2, H], bft)
            wp_b = gpool.tile([128, NKT // 2, H], bft)
            wc_sb = gpool.tile([128, 6, CP], bft)
            bc_sb = gpool.tile([CSL, 1], f32)
            mask_sb = gpool.tile([CSL, NP], f32)
            b2r_all = gpool.tile([128, K, NP], bft)
            ones_sb = gpool.tile([128, 1], bft)
            onesrow_sb = gpool.tile([1, 128], f32)
            selI_sb = gpool.tile([IPC, PL], bft)
            selJ_sb = gpool.tile([NE, PL], bft)

            hs_sb = persist.tile([128, 6, PL], bft)
            ts_sb = persist.tile([128, 6, PL], bft)

            with (
                tc.tile_pool(name="p1", bufs=1) as p1,
                tc.tile_pool(name="ps1", bufs=2, space="PSUM") as ps1,
            ):
                # ---------- input DMAs: sync queue in priority order ----------
                S_sb = p1.tile([120, 30], bft)
                nc.sync.dma_start(out=S_sb, in_=S_dr[:])
                att_b = p1.tile([128, NH, L], bft)
                nc.sync.dma_start(out=att_b, in_=att_gb[:])
                hpTt = p1.tile([IPC, E], bft)
                nc.gpsimd.dma_start(out=hpTt, in_=hpTt_in[:])
                tpTt = p1.tile([NE, E], bft)
                nc.gpsimd.dma_start(out=tpTt, in_=tpTt_in[:])
                seq_sb = p1.tile([128, 4, H], bft)
                nc.sync.dma_start(out=seq_sb, in_=seq_bf[:].rearrange("(a p) h -> p a h", p=128))
                nc.sync.dma_start(out=ones_sb, in_=ones_dr[:])
                nc.sync.dma_start(out=onesrow_sb, in_=onesrow_dr[:])
                nc.sync.dma_start(out=selJ_sb, in_=selJ_dr[:])
                wt2_sb = p1.tile([128, 6, E], bft)
                nc.sync.dma_start(out=wt2_sb, in_=wt2[:].rearrange("(a p) e -> p a e", p=128))
                nc.sync.dma_start(out=selI_sb, in_=selI_dr[:])
                bh_sb = p1.tile([128, 6], f32)
                nc.sync.dma_start(out=bh_sb, in_=bh_p[:])
                bt_sb = p1.tile([128, 6], f32)
                nc.sync.dma_start(out=bt_sb, in_=bt_t[:])
                wh2_sb = p1.tile([128, 6, E], bft)
                nc.sync.dma_start(out=wh2_sb, in_=wh2p[:].rearrange("(a p) e -> p a e", p=128))
                nc.sync.dma_start(out=bc_sb, in_=bc_sl[:])
                nc.sync.dma_start(
                    out=wp_a, in_=wp_sl[0 : KST // 2].rearrange("(a p) h -> p a h", p=128))
                nc.sync.dma_start(
                    out=wp_b, in_=wp_sl[KST // 2 : KST].rearrange("(a p) h -> p a h", p=128))
                nc.sync.dma_start(out=wc_sb, in_=wc_bf[:].rearrange("(a p) c -> p a c", p=128))
                nc.sync.dma_start(out=mask_sb, in_=mask_sl[:])
                # tanh-table warmup overlaps the input DMAs
                warm = p1.tile([1, 8], f32)
                nc.vector.memset(warm, 1.0)
                nc.scalar.activation(out=warm[0:1, 4:8], in_=warm[0:1, 0:4], func=AF.Tanh)

                # ---------- attention path: e_att^T packed in PSUM ----------
                eaPS = []
                ht_raw = p1.tile([128, 4, PL], bft)

                def emit_ea_mms(lt):
                    ea = ps1.tile([128, NH, 30], f32, tag="ea", bufs=3, name=f"ea{lt}")
                    eaPS.append(ea)
                    for h in range(NH):
                        nc.tensor.matmul(
                            ea[:, h, :], lhsT=att_b[0:120, h, 128 * lt : 128 * (lt + 1)],
                            rhs=S_sb[:], start=True, stop=True)

                def emit_products(lt):
                    # prod2 [l, h, il, j]: long inner runs; then halving
                    # add-tree over the 12 heads (reduce would need h inner)
                    eaI = p1.tile([128, NH, IPC], bft, tag="eaI", bufs=2)
                    nc.scalar.copy(out=eaI, in_=eaPS[lt][:, :, 24:30])
                    prod = p1.tile([128, NH, IPC, NE], bft, tag="prod", bufs=2)
                    in0 = (eaI[:, :, :].unsqueeze(3).broadcast_to([128, NH, IPC, NE]))
                    in1 = (eaPS[lt][:, :, 0:24].unsqueeze(2)
                           .broadcast_to([128, NH, IPC, NE]))
                    nc.vector.tensor_mul(out=prod, in0=in0, in1=in1)
                    t6 = p1.tile([128, 6, PL], bft, tag="t6", bufs=2)
                    nc.vector.tensor_add(
                        out=t6,
                        in0=prod[:, 0:6, :, :].rearrange("p h a b -> p h (a b)"),
                        in1=prod[:, 6:12, :, :].rearrange("p h a b -> p h (a b)"))
                    t3 = p1.tile([128, 3, PL], bft, tag="t3", bufs=2)
                    nc.vector.tensor_add(out=t3, in0=t6[:, 0:3, :], in1=t6[:, 3:6, :])
                    t1 = p1.tile([128, PL], bft, tag="t1", bufs=2)
                    nc.vector.tensor_add(out=t1, in0=t3[:, 0, :], in1=t3[:, 1, :])
                    nc.vector.tensor_add(out=t1, in0=t1, in1=t3[:, 2, :])
                    nc.vector.tensor_scalar_max(
                        out=ht_raw[:, lt, :], in0=t1, scalar1=0.0)

                with nc.allow_low_precision("bf16 pair-product tree; normalization is scale-invariant"):
                    for lt in range(4):
                        emit_ea_mms(lt)
                        emit_products(lt)

                    # ---------- normalization ----------
                    sum_ps = ps1.tile([1, PL], f32, tag="lsum", bufs=1)
                    for lt in range(4):
                        nc.tensor.matmul(sum_ps, lhsT=ones_sb[:], rhs=ht_raw[:, lt, :],
                                         start=(lt == 0), stop=(lt == 3))
                denom = p1.tile([1, PL], f32)
                nc.vector.tensor_scalar_add(out=denom, in0=sum_ps, scalar1=1e-10)
                recip = p1.tile([1, PL], f32)
                nc.vector.reciprocal(out=recip, in_=denom)
                rep_ps = ps1.tile([128, PL], f32, tag="acc", bufs=2)
                nc.tensor.matmul(rep_ps, lhsT=onesrow_sb[:], rhs=recip[:], start=True, stop=True)
                htn = p1.tile([128, 4, PL], bft)
                nc.vector.tensor_mul(
                    out=htn, in0=ht_raw,
                    in1=rep_ps[:].unsqueeze(1).broadcast_to([128, 4, PL]))

                # ---------- rs^T = seq^T @ ht_n ----------
                rsT = p1.tile([128, 6, PL], bft)
                for ht in range(6):
                    rp = ps1.tile([128, PL], f32, tag="acc", bufs=2)
                    for lt in range(4):
                        nc.tensor.matmul(rp, lhsT=seq_sb[:, lt, 128 * ht : 128 * (ht + 1)],
                                         rhs=htn[:, lt, :], start=(lt == 0), stop=(lt == 3))
                    nc.scalar.copy(out=rsT[:, ht, :], in_=rp)

                # ---------- extractors: ts first so its AllGather launches early ----------
                for Et in range(6):
                    ep = ps1.tile([128, PL], f32, tag="acc", bufs=2)
                    nc.tensor.matmul(ep, lhsT=tpTt[:, 128 * Et : 128 * (Et + 1)],
                                     rhs=selJ_sb[:], start=True, stop=False)
                    for ht in range(6):
                        nc.tensor.matmul(ep, lhsT=wt2_sb[:, ht, 128 * Et : 128 * (Et + 1)],
                                         rhs=rsT[:, ht, :], start=False, stop=(ht == 5))
                    nc.scalar.activation(out=ts_sb[:, Et, :], in_=ep, func=AF.Tanh,
                                         bias=bt_sb[:, Et : Et + 1])
                nc.scalar.dma_start(out=ts_cc_in[:], in_=ts_sb[:])
                nc.gpsimd.collective_compute(
                    "AllGather", ALU.bypass, replica_groups=groups,
                    ins=[ts_cc_in[:].opt()], outs=[ts_cc_o[:].opt()])

                for Et in range(6):
                    ep = ps1.tile([128, PL], f32, tag="acc", bufs=2)
                    nc.tensor.matmul(ep, lhsT=hpTt[:, 128 * Et : 128 * (Et + 1)],
                                     rhs=selI_sb[:], start=True, stop=False)
                    for ht in range(6):
                        nc.tensor.matmul(ep, lhsT=wh2_sb[:, ht, 128 * Et : 128 * (Et + 1)],
                                         rhs=rsT[:, ht, :], start=False, stop=(ht == 5))
                    nc.scalar.activation(out=hs_sb[:, Et, :], in_=ep, func=AF.Tanh,
                                         bias=bh_sb[:, Et : Et + 1])
                nc.scalar.dma_start(out=hs_cc_in[:], in_=hs_sb[:])
                nc.gpsimd.collective_compute(
                    "AllToAll", ALU.bypass, replica_groups=groups,
                    ins=[hs_cc_in[:].opt()], outs=[hs_cc_out[:].opt()])

                # b2r loads on the gpsimd queue: their descriptor generation
                # runs during the A2A instead of stalling the sync queue at
                # chunk-0 start.  j ascending so chunk ck's pair-blocks
                # (j = 2ck, 2ck+1) land first.  ts_cc_o[j, 64*k2+t, kk, pl]
                # holds ts[pair j*144+pl, e=(2*kk+k2)*64+t].
                for j in range(NCORE):
                    for h2 in range(2):
                        for k2 in range(2):
                            nc.gpsimd.dma_start(
                                out=b2r_all[64 * h2 : 64 * (h2 + 1), :, j * PL : (j + 1) * PL]
                                    .rearrange("p (kk k2) c -> p k2 kk c", k2=2)[:, k2],
                                in_=ts_cc_o[j, 64 * k2 : 64 * (k2 + 1), :, :],
                            )

            # ---------- phase 2: feature + classifier over pair chunks ----------
            with (
                tc.tile_pool(name="p2", bufs=2) as p2,
                tc.tile_pool(name="ps2", bufs=1, space="PSUM") as ps2,
            ):
                def emit_hsb_bl(ck, k):
                    # hs rows for (k, all 4 sp) broadcast to 64 partitions;
                    # descriptor generation split across two queues so hsb
                    # production keeps ahead of the PE (one gen is ~0.65 us)
                    hsb = p2.tile([128, 4, CW], bft, tag="hsb", bufs=4)
                    for half in range(2):
                        eng = nc.sync if half == 0 else nc.scalar
                        for jb in range(2):
                            eng.dma_start(
                                out=hsb[64 * half : 64 * (half + 1), :, jb * PL : (jb + 1) * PL],
                                in_=bass.AP(
                                    tensor=hs_cc_out,
                                    offset=(16 * (2 * ck + jb) + 2 * half + k // 6) * (6 * PL)
                                           + (k % 6) * PL,
                                    ap=[[0, 64], [4 * 6 * PL, 4], [1, PL]],
                                ),
                            )
                    bl = p2.tile([128, 4, CW], bft, tag="bl", bufs=16,
                                 name=f"bl{ck}_{k}")
                    nc.vector.tensor_mul(
                        out=bl, in0=hsb,
                        in1=b2r_all[:, k, ck * CW : (ck + 1) * CW]
                            .unsqueeze(1).broadcast_to([128, 4, CW]))
                    return bl

                def emit_mms(fps, k, bl):
                    for h in range(6):
                        for sp in range(4):
                            kt = k * 4 + sp
                            wt = wp_a if kt < NKT // 2 else wp_b
                            nc.tensor.matmul(
                                fps[h], lhsT=wt[:, kt % (NKT // 2), 128 * h : 128 * (h + 1)],
                                rhs=bl[:, sp, :],
                                start=(k == 0 and sp == 0),
                                stop=(k == K - 1 and sp == 3))

                def emit_tail(ck):
                    # post-RS bias+mask; emitted one chunk late so the RS wait
                    # overlaps the next chunk's compute
                    lgf_b = p2.tile([CSL, CW], bft, tag="lgfb", bufs=2, name=f"lgfb{ck}")
                    nc.gpsimd.dma_start(out=lgf_b, in_=lg_cc_out[ck, :, :])
                    lgf = p2.tile([CSL, CW], f32, tag="lgf", bufs=2, name=f"lgf{ck}")
                    nc.vector.tensor_scalar_add(out=lgf, in0=lgf_b, scalar1=bc_sb[:])
                    nc.vector.tensor_mul(
                        out=lgf, in0=lgf, in1=mask_sb[:, ck * CW : (ck + 1) * CW])
                    nc.gpsimd.dma_start(
                        out=out_lgT[:, ck * CW : (ck + 1) * CW], in_=lgf[:])

                for ck in range(NCHUNK):
                    fps = []
                    for h in range(6):
                        fps.append(ps2.tile([128, CW], f32, tag=f"feat{h}", bufs=1,
                                            name=f"fps{h}"))
                    if ck == 0:
                        # chunk 0: k-interleaved so matmuls start as soon as
                        # the first bl is ready after the A2A
                        for k in range(K):
                            bl = emit_hsb_bl(ck, k)
                            emit_mms(fps, k, bl)
                    else:
                        bls = [emit_hsb_bl(ck, k) for k in range(K)]
                        if ck >= 1:
                            emit_tail(ck - 1)
                        for h in range(6):
                            for k in range(K):
                                for sp in range(4):
                                    kt = k * 4 + sp
                                    wt = wp_a if kt < NKT // 2 else wp_b
                                    nc.tensor.matmul(
                                        fps[h], lhsT=wt[:, kt % (NKT // 2), 128 * h : 128 * (h + 1)],
                                        rhs=bls[k][:, sp, :],
                                        start=(k == 0 and sp == 0),
                                        stop=(k == K - 1 and sp == 3))
                    lgp = ps2.tile([CP, CW], f32, tag="lgp", bufs=1)
                    for h in range(6):
                        fT = p2.tile([128, CW], bft, tag="fT", bufs=2)
                        nc.scalar.copy(out=fT, in_=fps[h])
                        nc.tensor.matmul(lgp, lhsT=wc_sb[:, h, :], rhs=fT,
                                         start=(h == 0), stop=(h == 5))
                    lg_sl = p2.tile([CP, CW], bft, tag="lgsl", bufs=2)
                    nc.scalar.copy(out=lg_sl, in_=lgp)
                    nc.scalar.dma_start(out=lg_cc_in[ck, :, :], in_=lg_sl[:])
                    nc.gpsimd.collective_compute(
                        "ReduceScatter", ALU.add, replica_groups=groups,
                        ins=[lg_cc_in[ck, :, :].opt()],
                        outs=[lg_cc_out[ck, :, :].opt()])
                emit_tail(NCHUNK - 1)

    if not nc.is_finalized():
        nc.finalize()
    return nc


_NC_CACHE = None


def kernel(**inputs):
    global _NC_CACHE
    from concourse.bass_utils import run_bass_kernel_spmd

    if _NC_CACHE is None:
        _NC_CACHE = build_bass()
    in_maps = _host_prep(inputs)
    res = run_bass_kernel_spmd(_NC_CACHE, in_maps, core_ids=list(range(NCORE)))
    kernel.last_results = res
    full = np.concatenate(
        [np.asarray(res.results[c]["out_lgT"]) for c in range(NCORE)], axis=0
    )[:C]  # [97, 1152]
    return np.ascontiguousarray(full.T).astype(np.float32)


# revision 48
# speedup vs baseline: 1.3229x; 1.3229x over previous
"""Trainium2 Bass kernel for nn_DocREModel_Triangle (DocRE block-bilinear model).

Strategy (8 NeuronCores, single SPMD NEFF):
  Phase 1 (pair-parallel): core c owns batch b=c//4 and entity rows
  i in [6*(c%4), 6*(c%4)+6) -> 144 (i,j) pairs. Computes, in a transposed
  layout (feature dim on partitions, pairs on the free dim): mention
  gather + logsumexp entity embeddings, attention-row gather + mention-sum
  (fused into selector matmuls whose outputs stay packed in PSUM),
  pairwise head-products (DVE mul + halving add-tree over heads),
  l-normalization, context vectors rs^T, and the two tanh extractors
  hs^T/ts^T [768, 144] bf16. The hpart/tpart broadcast-adds are folded
  into the extractor PSUM accumulation as tiny selector matmuls.
  Collectives: AllGather replicates ts^T, AllToAll redistributes hs^T by
  s-slices. Both cc buffers are partition-major [128, 6, 144] so the
  bounce DMAs are straight copies; a host-side column permutation of Wh
  makes the A2A 16-partition shards line up with s-slices.
  Phase 2 (contraction-parallel): core c holds Wp rows (k, s in [8c,8c+8),
  t) resident in SBUF (bf16, 9.4 MB) and accumulates feature^T over its
  6144 (k,s,t) rows for all 1152 pairs in 4 pair-chunks of 288. The hs
  operand is replicated to 64 partitions by stride-0-partition DMA, ts is
  loaded once [128, 12, 1152] during the A2A. Matmuls run sp-inner so 4
  consecutive MMs accumulate into the same PSUM bank. The classifier is
  C-padded to 104 and ReduceScattered: core c keeps logit rows
  [13c, 13c+13); the host concatenates the 8 slices.
"""

import numpy as np
import ml_dtypes

bf16 = ml_dtypes.bfloat16

B, L, H, NH = 2, 512, 768, 12
NE, NM = 24, 4
E, BS, C = 768, 64, 97
K = E // BS                      # 12 blocks
NCORE = 8
IPC = NE // 4                    # 6 i-rows per core (4 cores per batch elem)
PL = IPC * NE                    # 144 local pairs
NP = B * NE * NE                 # 1152 global pairs
SL = 64 // NCORE                 # 8 s-values per core
KST = K * SL * BS                # 6144 contraction rows per core
NKT = KST // 128                 # 48 contraction tiles
NCHUNK = 4                       # pair chunks of 288 (= 2 pair-blocks)
CW = NP // NCHUNK                # 288
CP = 104                         # class count padded to a multiple of 8
CSL = CP // NCORE                # 13 logit rows per core after ReduceScatter

# hs column permutation: extractor output column c = Et*128 + p0 holds
# original dim e = k*64 + s with k = 6*(p0%2) + Et, s = 8*(p0//16) +
# (p0%16)//2.  The p-major cc buffer [128, 6, PL] then has A2A shard j
# (partitions 16j..16j+16) == s-slice [8j, 8j+8), and the phase-2 hs
# broadcast reads are 3-dim APs.
_PERM2 = np.empty(E, dtype=np.int64)
for _c in range(E):
    _p0, _Et = _c % 128, _c // 128
    _k = 6 * (_p0 % 2) + _Et
    _s = 8 * (_p0 // 16) + (_p0 % 16) // 2
    _PERM2[_c] = _k * 64 + _s


def _host_prep(inputs):
    """Build the 8 per-core input maps from the full inputs."""
    seq = np.ascontiguousarray(inputs["sequence_output"], dtype=np.float32)
    att = np.ascontiguousarray(inputs["attention"], dtype=np.float32)
    Wh = np.asarray(inputs["Wh"], dtype=np.float32)
    bh = np.asarray(inputs["bh"], dtype=np.float32)
    Wt = np.asarray(inputs["Wt"], dtype=np.float32)
    bt = np.asarray(inputs["bt"], dtype=np.float32)
    Wp = np.asarray(inputs["Wp"], dtype=np.float32)
    Wc = np.asarray(inputs["Wc"], dtype=np.float32)
    bc = np.asarray(inputs["bc"], dtype=np.float32)
    mpos = np.asarray(inputs["mention_pos"]).astype(np.int64)

    wh1p = np.ascontiguousarray(Wh[:H][:, _PERM2].astype(bf16))
    wh2p = np.ascontiguousarray(Wh[H:][:, _PERM2].astype(bf16))
    wt1 = np.ascontiguousarray(Wt[:H].astype(bf16))
    wt2 = np.ascontiguousarray(Wt[H:].astype(bf16))
    bh_p = np.ascontiguousarray(bh[_PERM2].reshape(6, 128).T.astype(np.float32))
    bt_t = np.ascontiguousarray(bt.reshape(6, 128).T.astype(np.float32))
    wc_pad = np.zeros((H, CP), dtype=np.float32)
    wc_pad[:, :C] = Wc
    wc_bf = np.ascontiguousarray(wc_pad.astype(bf16))
    bc_pad = np.zeros((CP,), dtype=np.float32)
    bc_pad[:C] = bc
    wp4 = Wp.reshape(K, 64, BS, H)

    # per-pair mask row (zero for self-pairs i == j)
    mrow = np.ones((NP,), dtype=np.float32)
    for c in range(NCORE):
        for il in range(IPC):
            ig = (c % 4) * IPC + il
            mrow[c * PL + il * NE + ig] = 0.0

    in_maps = []
    for c in range(NCORE):
        b = c // 4
        i0 = (c % 4) * IPC
        ents = list(range(NE)) + list(range(i0, i0 + IPC))  # 24 j-side + 6 i-side

        # mention logsumexp + first-layer projections on host (tiny: 56M MACs)
        ee = np.log(np.exp(seq[b][mpos[b][ents]].astype(np.float64)).sum(axis=1))
        hpTt_in = np.ascontiguousarray(
            (ee[NE:] @ Wh[:H].astype(np.float64))[:, _PERM2].astype(bf16))
        tpTt_in = np.ascontiguousarray(
            (ee[:NE] @ Wt[:H].astype(np.float64)).astype(bf16))

        # host-gathered attention rows: [128, NH, L] bf16,
        # rows = 4m x 30ents (+ 8 pad) per head
        ai = np.zeros((128, NH), dtype=np.int32)
        for h in range(NH):
            for m in range(NM):
                for e_i, ent in enumerate(ents):
                    ai[m * 30 + e_i, h] = h * L + mpos[b, ent, m]
        att_gb = np.ascontiguousarray(
            att[b].reshape(NH * L, L)[ai.T].transpose(1, 0, 2).astype(bf16))

        wp_sl = np.ascontiguousarray(
            wp4[:, SL * c : SL * (c + 1)].reshape(KST, H).astype(bf16)
        )

        in_maps.append(
            {
                "seq_bf": np.ascontiguousarray(seq[b].astype(bf16)),
                "att_gb": att_gb,
                "hpTt_in": hpTt_in,
                "tpTt_in": tpTt_in,
                "wh2p": wh2p,
                "wt2": wt2,
                "bh_p": bh_p,
                "bt_t": bt_t,
                "wp_sl": wp_sl,
                "wc_bf": wc_bf,
                "bc_sl": np.ascontiguousarray(
                    bc_pad[CSL * c : CSL * (c + 1)].reshape(CSL, 1)),
                "mask_sl": np.ascontiguousarray(
                    np.tile(mrow[None, :], (CSL, 1))),
            }
        )
    return in_maps


def _build_consts():
    # selectors summing the mention rows per entity during the attention /
    # mention transpose-matmuls (the /4 mean cancels in the normalization)
    S = np.zeros((120, 30), dtype=bf16)
    for m in range(NM):
        for e_i in range(30):
            S[m * 30 + e_i, e_i] = 1.0
    S2 = np.zeros((128, 30), dtype=bf16)
    for m in range(NM):
        for e_i in range(30):
            S2[m * 32 + e_i, e_i] = 1.0
    ones_bf = np.ones((128, 1), dtype=bf16)
    ones_row = np.ones((1, 128), dtype=np.float32)
    # pair selectors: selI picks hpart row il for pair (il, j), selJ picks
    # tpart row j; used as 6-/24-deep matmuls folded into the extractors
    selI = np.zeros((IPC, PL), dtype=bf16)
    selJ = np.zeros((NE, PL), dtype=bf16)
    for il in range(IPC):
        for j in range(NE):
            selI[il, il * NE + j] = 1.0
            selJ[j, il * NE + j] = 1.0
    return S, S2, ones_bf, ones_row, selI, selJ


def build_bass():
    import concourse.bass as bass
    import concourse.mybir as mybir
    import concourse.tile as tile
    from concourse.bacc import Bacc

    f32 = mybir.dt.float32
    bft = mybir.dt.bfloat16
    i32 = mybir.dt.int32
    AF = mybir.ActivationFunctionType
    ALU = mybir.AluOpType

    nc = Bacc("TRN2", num_devices=NCORE)

    # ---- I/O ----
    seq_bf = nc.dram_tensor("seq_bf", [L, H], bft, kind="ExternalInput")
    att_gb = nc.dram_tensor("att_gb", [128, NH, L], bft, kind="ExternalInput")
    hpTt_in = nc.dram_tensor("hpTt_in", [IPC, E], bft, kind="ExternalInput")
    tpTt_in = nc.dram_tensor("tpTt_in", [NE, E], bft, kind="ExternalInput")
    wh2p = nc.dram_tensor("wh2p", [H, E], bft, kind="ExternalInput")
    wt2 = nc.dram_tensor("wt2", [H, E], bft, kind="ExternalInput")
    bh_p = nc.dram_tensor("bh_p", [128, 6], f32, kind="ExternalInput")
    bt_t = nc.dram_tensor("bt_t", [128, 6], f32, kind="ExternalInput")
    wp_sl = nc.dram_tensor("wp_sl", [KST, H], bft, kind="ExternalInput")
    wc_bf = nc.dram_tensor("wc_bf", [H, CP], bft, kind="ExternalInput")
    bc_sl = nc.dram_tensor("bc_sl", [CSL, 1], f32, kind="ExternalInput")
    mask_sl = nc.dram_tensor("mask_sl", [CSL, NP], f32, kind="ExternalInput")
    out_lgT = nc.dram_tensor("out_lgT", [CSL, NP], f32, kind="ExternalOutput")

    S_np, S2_np, ones_np, onesrow_np, selI_np, selJ_np = _build_consts()
    S_dr = nc.inline_tensor(S_np, "sel_const")
    S2_dr = nc.inline_tensor(S2_np, "s2_const")
    ones_dr = nc.inline_tensor(ones_np, "ones_const")
    onesrow_dr = nc.inline_tensor(onesrow_np, "onesrow_const")
    selI_dr = nc.inline_tensor(selI_np, "selI_const")
    selJ_dr = nc.inline_tensor(selJ_np, "selJ_const")

    # collective buffers, partition-major so bounce DMAs are straight copies
    hs_cc_in = nc.dram_tensor("hs_cc_in", [128, 6, PL], bft)
    hs_cc_out = nc.dram_tensor("hs_cc_out", [128, 6, PL], bft)
    ts_cc_in = nc.dram_tensor("ts_cc_in", [128, 6, PL], bft)
    ts_cc_o = nc.dram_tensor("ts_cc_o", [NCORE, 128, 6, PL], bft, addr_space="Shared")
    lg_cc_in = nc.dram_tensor("lg_cc_in", [NCHUNK, CP, CW], bft)
    lg_cc_out = nc.dram_tensor("lg_cc_out", [NCHUNK, CSL, CW], bft)
    groups = [list(range(NCORE))]

    with tile.TileContext(nc) as tc:
        with (
            tc.tile_pool(name="gpool", bufs=1) as gpool,
            tc.tile_pool(name="persist", bufs=1) as persist,
        ):
            # ---------- whole-kernel-lifetime weights / constants ----------
            # wp split in two so the lhsT partition pitch stays under 64 KiB
            # (a single [128, 48, 768] bf16 tile has 73.7 KB pitch, which
            # disables the fast-weight-load path: 108 ns vs 62 ns LDWEIGHTS)
            wp_a = gpool.tile([128, NKT // 2, H], bft)
            wp_b = gpool.tile([128, NKT // 2, H], bft)
            wc_sb = gpool.tile([128, 6, CP], bft)
            bc_sb = gpool.tile([CSL, 1], f32)
            mask_sb = gpool.tile([CSL, NP], f32)
            b2r_all = gpool.tile([128, K, NP], bft)
            ones_sb = gpool.tile([128, 1], bft)
            onesrow_sb = gpool.tile([1, 128], f32)
            selI_sb = gpool.tile([IPC, PL], bft)
            selJ_sb = gpool.tile([NE, PL], bft)

            hs_sb = persist.tile([128, 6, PL], bft)
            ts_sb = persist.tile([128, 6, PL], bft)

            with (
                tc.tile_pool(name="p1", bufs=1) as p1,
                tc.tile_pool(name="ps1", bufs=2, space="PSUM") as ps1,
            ):
                # ---------- input DMAs: sync queue in priority order ----------
                S_sb = p1.tile([120, 30], bft)
                nc.sync.dma_start(out=S_sb, in_=S_dr[:])
                att_b = p1.tile([128, NH, L], bft)
                nc.sync.dma_start(out=att_b, in_=att_gb[:])
                hpTt = p1.tile([IPC, E], bft)
                nc.gpsimd.dma_start(out=hpTt, in_=hpTt_in[:])
                tpTt = p1.tile([NE, E], bft)
                nc.gpsimd.dma_start(out=tpTt, in_=tpTt_in[:])
                seq_sb = p1.tile([128, 4, H], bft)
                nc.sync.dma_start(out=seq_sb, in_=seq_bf[:].rearrange("(a p) h -> p a h", p=128))
                nc.sync.dma_start(out=ones_sb, in_=ones_dr[:])
                nc.sync.dma_start(out=onesrow_sb, in_=onesrow_dr[:])
                nc.sync.dma_start(out=selJ_sb, in_=selJ_dr[:])
                wt2_sb = p1.tile([128, 6, E], bft)
                nc.sync.dma_start(out=wt2_sb, in_=wt2[:].rearrange("(a p) e -> p a e", p=128))
                nc.sync.dma_start(out=selI_sb, in_=selI_dr[:])
                bh_sb = p1.tile([128, 6], f32)
                nc.sync.dma_start(out=bh_sb, in_=bh_p[:])
                bt_sb = p1.tile([128, 6], f32)
                nc.sync.dma_start(out=bt_sb, in_=bt_t[:])
                wh2_sb = p1.tile([128, 6, E], bft)
                nc.sync.dma_start(out=wh2_sb, in_=wh2p[:].rearrange("(a p) e -> p a e", p=128))
                nc.sync.dma_start(out=bc_sb, in_=bc_sl[:])
                nc.sync.dma_start(
                    out=wp_a, in_=wp_sl[0 : KST // 2].rearrange("(a p) h -> p a h", p=128))
                nc.sync.dma_start(
                    out=wp_b, in_=wp_sl[KST // 2 : KST].rearrange("(a p) h -> p a h", p=128))
                nc.sync.dma_start(out=wc_sb, in_=wc_bf[:].rearrange("(a p) c -> p a c", p=128))
                nc.sync.dma_start(out=mask_sb, in_=mask_sl[:])
                # tanh-table warmup overlaps the input DMAs
                warm = p1.tile([1, 8], f32)
                nc.vector.memset(warm, 1.0)
                nc.scalar.activation(out=warm[0:1, 4:8], in_=warm[0:1, 0:4], func=AF.Tanh)

                # ---------- attention path: e_att^T packed in PSUM ----------
                eaPS = []
                ht_raw = p1.tile([128, 4, PL], bft)

                def emit_ea_mms(lt):
                    ea = ps1.tile([128, NH, 30], f32, tag="ea", bufs=3, name=f"ea{lt}")
                    eaPS.append(ea)
                    for h in range(NH):
                        nc.tensor.matmul(
                            ea[:, h, :], lhsT=att_b[0:120, h, 128 * lt : 128 * (lt + 1)],
                            rhs=S_sb[:], start=True, stop=True)

                def emit_products(lt):
                    # prod2 [l, h, il, j]: long inner runs; then halving
                    # add-tree over the 12 heads (reduce would need h inner)
                    eaI = p1.tile([128, NH, IPC], bft, tag="eaI", bufs=2)
                    nc.scalar.copy(out=eaI, in_=eaPS[lt][:, :, 24:30])
                    prod = p1.tile([128, NH, IPC, NE], bft, tag="prod", bufs=2)
                    in0 = (eaI[:, :, :].unsqueeze(3).broadcast_to([128, NH, IPC, NE]))
                    in1 = (eaPS[lt][:, :, 0:24].unsqueeze(2)
                           .broadcast_to([128, NH, IPC, NE]))
                    nc.vector.tensor_mul(out=prod, in0=in0, in1=in1)
                    t6 = p1.tile([128, 6, PL], bft, tag="t6", bufs=2)
                    nc.vector.tensor_add(
                        out=t6,
                        in0=prod[:, 0:6, :, :].rearrange("p h a b -> p h (a b)"),
                        in1=prod[:, 6:12, :, :].rearrange("p h a b -> p h (a b)"))
                    t3 = p1.tile([128, 3, PL], bft, tag="t3", bufs=2)
                    nc.vector.tensor_add(out=t3, in0=t6[:, 0:3, :], in1=t6[:, 3:6, :])
                    t1 = p1.tile([128, PL], bft, tag="t1", bufs=2)
                    nc.vector.tensor_add(out=t1, in0=t3[:, 0, :], in1=t3[:, 1, :])
                    nc.vector.tensor_add(out=t1, in0=t1, in1=t3[:, 2, :])
                    nc.vector.tensor_scalar_max(
                        out=ht_raw[:, lt, :], in0=t1, scalar1=0.0)

                with nc.allow_low_precision("bf16 pair-product tree; normalization is scale-invariant"):
                    for lt in range(4):
                        emit_ea_mms(lt)
                        emit_products(lt)

                    # ---------- normalization ----------
                    sum_ps = ps1.tile([1, PL], f32, tag="lsum", bufs=1)
                    for lt in range(4):
                        nc.tensor.matmul(sum_ps, lhsT=ones_sb[:], rhs=ht_raw[:, lt, :],
                                         start=(lt == 0), stop=(lt == 3))
                denom = p1.tile([1, PL], f32)
                nc.vector.tensor_scalar_add(out=denom, in0=sum_ps, scalar1=1e-10)
                recip = p1.tile([1, PL], f32)
                nc.vector.reciprocal(out=recip, in_=denom)
                rep_ps = ps1.tile([128, PL], f32, tag="acc", bufs=2)
                nc.tensor.matmul(rep_ps, lhsT=onesrow_sb[:], rhs=recip[:], start=True, stop=True)
                htn = p1.tile([128, 4, PL], bft)
                nc.vector.tensor_mul(
                    out=htn, in0=ht_raw,
                    in1=rep_ps[:].unsqueeze(1).broadcast_to([128, 4, PL]))

                # ---------- rs^T = seq^T @ ht_n ----------
                rsT = p1.tile([128, 6, PL], bft)
                for ht in range(6):
                    rp = ps1.tile([128, PL], f32, tag="acc", bufs=2)
                    for lt in range(4):
                        nc.tensor.matmul(rp, lhsT=seq_sb[:, lt, 128 * ht : 128 * (ht + 1)],
                                         rhs=htn[:, lt, :], start=(lt == 0), stop=(lt == 3))
                    nc.scalar.copy(out=rsT[:, ht, :], in_=rp)

                # ---------- extractors: ts first so its AllGather launches early ----------
                for Et in range(6):
                    ep = ps1.tile([128, PL], f32, tag="acc", bufs=2)
                    nc.tensor.matmul(ep, lhsT=tpTt[:, 128 * Et : 128 * (Et + 1)],
                                     rhs=selJ_sb[:], start=True, stop=False)
                    for ht in range(6):
                        nc.tensor.matmul(ep, lhsT=wt2_sb[:, ht, 128 * Et : 128 * (Et + 1)],
                                         rhs=rsT[:, ht, :], start=False, stop=(ht == 5))
                    nc.scalar.activation(out=ts_sb[:, Et, :], in_=ep, func=AF.Tanh,
                                         bias=bt_sb[:, Et : Et + 1])
                nc.scalar.dma_start(out=ts_cc_in[:], in_=ts_sb[:])
                nc.gpsimd.collective_compute(
                    "AllGather", ALU.bypass, replica_groups=groups,
                    ins=[ts_cc_in[:].opt()], outs=[ts_cc_o[:].opt()])

                for Et in range(6):
                    ep = ps1.tile([128, PL], f32, tag="acc", bufs=2)
                    nc.tensor.matmul(ep, lhsT=hpTt[:, 128 * Et : 128 * (Et + 1)],
                                     rhs=selI_sb[:], start=True, stop=False)
                    for ht in range(6):
                        nc.tensor.matmul(ep, lhsT=wh2_sb[:, ht, 128 * Et : 128 * (Et + 1)],
                                         rhs=rsT[:, ht, :], start=False, stop=(ht == 5))
                    nc.scalar.activation(out=hs_sb[:, Et, :], in_=ep, func=AF.Tanh,
                                         bias=bh_sb[:, Et : Et + 1])
                nc.scalar.dma_start(out=hs_cc_in[:], in_=hs_sb[:])
                nc.gpsimd.collective_compute(
                    "AllToAll", ALU.bypass, replica_groups=groups,
                    ins=[hs_cc_in[:].opt()], outs=[hs_cc_out[:].opt()])

                # b2r loads on the gpsimd queue: their descriptor generation
                # runs during the A2A instead of stalling the sync queue at
                # chunk-0 start.  j ascending so chunk ck's pair-blocks
                # (j = 2ck, 2ck+1) land first.  ts_cc_o[j, 64*k2+t, kk, pl]
                # holds ts[pair j*144+pl, e=(2*kk+k2)*64+t].  j >= 4 (needed
                # only from chunk 2) is deferred past chunk-0's gpsimd hsb
                # gens.
                for j in range(4):
                    for h2 in range(2):
                        for k2 in range(2):
                            nc.gpsimd.dma_start(
                                out=b2r_all[64 * h2 : 64 * (h2 + 1), :, j * PL : (j + 1) * PL]
                                    .rearrange("p (kk k2) c -> p k2 kk c", k2=2)[:, k2],
                                in_=ts_cc_o[j, 64 * k2 : 64 * (k2 + 1), :, :],
                            )

            # ---------- phase 2: feature + classifier over pair chunks ----------
            with (
                tc.tile_pool(name="p2", bufs=2) as p2,
                tc.tile_pool(name="ps2", bufs=1, space="PSUM") as ps2,
            ):
                def emit_hsb_bl(ck, k, gps=False):
                    # hs rows for (k, all 4 sp) broadcast to 64 partitions;
                    # descriptor generation split across queues so hsb
                    # production keeps ahead of the PE (one gen is ~0.65 us)
                    hsb = p2.tile([128, 4, CW], bft, tag="hsb", bufs=4)
                    for half in range(2):
                        eng = nc.gpsimd if gps else (nc.sync if half == 0 else nc.scalar)
                        for jb in range(2):
                            eng.dma_start(
                                out=hsb[64 * half : 64 * (half + 1), :, jb * PL : (jb + 1) * PL],
                                in_=bass.AP(
                                    tensor=hs_cc_out,
                                    offset=(16 * (2 * ck + jb) + 2 * half + k // 6) * (6 * PL)
                                           + (k % 6) * PL,
                                    ap=[[0, 64], [4 * 6 * PL, 4], [1, PL]],
                                ),
                            )
                    bl = p2.tile([128, 4, CW], bft, tag="bl", bufs=16,
                                 name=f"bl{ck}_{k}")
                    nc.vector.tensor_mul(
                        out=bl, in0=hsb,
                        in1=b2r_all[:, k, ck * CW : (ck + 1) * CW]
                            .unsqueeze(1).broadcast_to([128, 4, CW]))
                    return bl

                def emit_mms(fps, k, bl):
                    for h in range(6):
                        for sp in range(4):
                            kt = k * 4 + sp
                            wt = wp_a if kt < NKT // 2 else wp_b
                            nc.tensor.matmul(
                                fps[h], lhsT=wt[:, kt % (NKT // 2), 128 * h : 128 * (h + 1)],
                                rhs=bl[:, sp, :],
                                start=(k == 0 and sp == 0),
                                stop=(k == K - 1 and sp == 3))

                def emit_tail(ck):
                    # post-RS bias+mask; emitted one chunk late so the RS wait
                    # overlaps the next chunk's compute
                    lgf_b = p2.tile([CSL, CW], bft, tag="lgfb", bufs=2, name=f"lgfb{ck}")
                    nc.gpsimd.dma_start(out=lgf_b, in_=lg_cc_out[ck, :, :])
                    lgf = p2.tile([CSL, CW], f32, tag="lgf", bufs=2, name=f"lgf{ck}")
                    nc.vector.tensor_scalar_add(out=lgf, in0=lgf_b, scalar1=bc_sb[:])
                    nc.vector.tensor_mul(
                        out=lgf, in0=lgf, in1=mask_sb[:, ck * CW : (ck + 1) * CW])
                    nc.gpsimd.dma_start(
                        out=out_lgT[:, ck * CW : (ck + 1) * CW], in_=lgf[:])

                for ck in range(NCHUNK):
                    fps = []
                    for h in range(6):
                        fps.append(ps2.tile([128, CW], f32, tag=f"feat{h}", bufs=1,
                                            name=f"fps{h}"))
                    if ck == 0:
                        # chunk 0: k-interleaved so matmuls start as soon as
                        # the first bl is ready after the A2A; the last 4 k
                        # generate on gpsimd (a third gen queue)
                        for k in range(K):
                            bl = emit_hsb_bl(ck, k, gps=(k >= 8))
                            emit_mms(fps, k, bl)
                        # deferred b2r pieces (chunks 2-3) behind ck0's gens
                        for j in range(4, NCORE):
                            for h2 in range(2):
                                for k2 in range(2):
                                    nc.gpsimd.dma_start(
                                        out=b2r_all[64 * h2 : 64 * (h2 + 1), :, j * PL : (j + 1) * PL]
                                            .rearrange("p (kk k2) c -> p k2 kk c", k2=2)[:, k2],
                                        in_=ts_cc_o[j, 64 * k2 : 64 * (k2 + 1), :, :],
                                    )
                    else:
                        bls = [emit_hsb_bl(ck, k) for k in range(K)]
                        if ck >= 1:
                            emit_tail(ck - 1)
                        for h in range(6):
                            for k in range(K):
                                for sp in range(4):
                                    kt = k * 4 + sp
                                    wt = wp_a if kt < NKT // 2 else wp_b
                                    nc.tensor.matmul(
                                        fps[h], lhsT=wt[:, kt % (NKT // 2), 128 * h : 128 * (h + 1)],
                                        rhs=bls[k][:, sp, :],
                                        start=(k == 0 and sp == 0),
                                        stop=(k == K - 1 and sp == 3))
                    lgp = ps2.tile([CP, CW], f32, tag="lgp", bufs=1)
                    for h in range(6):
                        fT = p2.tile([128, CW], bft, tag="fT", bufs=2)
                        nc.scalar.copy(out=fT, in_=fps[h])
                        nc.tensor.matmul(lgp, lhsT=wc_sb[:, h, :], rhs=fT,
                                         start=(h == 0), stop=(h == 5))
                    lg_sl = p2.tile([CP, CW], bft, tag="lgsl", bufs=2)
                    nc.scalar.copy(out=lg_sl, in_=lgp)
                    nc.scalar.dma_start(out=lg_cc_in[ck, :, :], in_=lg_sl[:])
                    nc.gpsimd.collective_compute(
                        "ReduceScatter", ALU.add, replica_groups=groups,
                        ins=[lg_cc_in[ck, :, :].opt()],
                        outs=[lg_cc_out[ck, :, :].opt()])
                emit_tail(NCHUNK - 1)

    if not nc.is_finalized():
        nc.finalize()
    return nc


_NC_CACHE = None


def kernel(**inputs):
    global _NC_CACHE
    from concourse.bass_utils import run_bass_kernel_spmd

    if _NC_CACHE is None:
        _NC_CACHE = build_bass()
    in_maps = _host_prep(inputs)
    res = run_bass_kernel_spmd(_NC_CACHE, in_maps, core_ids=list(range(NCORE)))
    kernel.last_results = res
    full = np.concatenate(
        [np.asarray(res.results[c]["out_lgT"]) for c in range(NCORE)], axis=0
    )[:C]  # [97, 1152]
    return np.ascontiguousarray(full.T).astype(np.float32)
